# revision 14
# baseline (speedup 1.0000x reference)
"""Dynamic per-pixel 3x3 filtering on 8 Trainium2 NeuronCores.

out[b,c,y,x] = sum_{ki,kj} img[b,c,y+ki-1,x+kj-1] * kernels[b,c,ki*3+kj,y,x]
(zero padding outside the image).

Sharding: pure data parallel, one batch sample per core (B=8, 8 cores).

Per-core layout: partition p holds 4 CONSECUTIVE image rows 4p..4p+3
(8 KB contiguous per partition -> single-descriptor DMAs). A +-1 row
shift is then a FREE-DIM shift inside an extended tile
ext[p, bb, xx] = img[4p+bb-1, xx-1] (6 rows x 514 cols, zero padded).
The two boundary rows (4p-1, 4p+4) live on the neighbouring partition,
so they are produced by the otherwise-idle TensorE as a +-1 partition
shift: a matmul with a shifted identity as the stationary operand
(edge partitions zero-fill automatically), evacuated PSUM->SBUF by ACT
with a free f32->fp16 cast. This keeps img HBM traffic at exactly
1 MB/channel instead of re-reading shifted rows.

All elementwise work runs on DVE in fp16: TensorTensor supports the
2x_1p perf mode only when every operand is a packed 2-byte dtype,
doubling throughput vs f32 (measured 2287 -> 1221 ns per [128,2048]
pass). fp16 chain accumulation keeps max rel err ~1.2e-3, well under
the 2e-2 gate. ACT does the f32->fp16 casts of the streamed kernel
taps.

The DMA fleet is HBM-bound (~22 GB/s per SDMA engine x16), so traffic
is minimal and spread: kernel taps stream as nine 1 MB single-
descriptor-per-partition DMAs per channel on the SP HWDGE ring, img on
the ACT HWDGE ring, stores on the gpsimd SWDGE ring. Output is stored
fp16 (halves store traffic), widened on host. The last channel's final
tap and store are split in half so the post-last-DMA pipeline drain
(cast -> mult -> add -> store) runs at half tile size.
"""

from contextlib import ExitStack

import numpy as np

import concourse.bacc as bacc
import concourse.mybir as mybir
import concourse.tile as tile
from concourse import masks
from concourse.bass_utils import run_bass_kernel_spmd

C, H, W = 3, 512, 512
KK = 9
NCORES = 8
P = 128
RPB = H // P         # 4 rows per partition
FW = RPB * W         # 2048 free-dim elems of a channel tile
EXT_W = W + 2        # 514: row length incl. zero pad cols
F32 = mybir.dt.float32
F16 = mybir.dt.float16


def _r3(ap):
    """[128, n*W] -> [128, n, W] row-block view."""
    return ap.rearrange("p (b x) -> p b x", x=W)


def _emit(nc, tc, ctx):
    img = nc.dram_tensor("img", (C, H, W), F32, kind="ExternalInput").ap()
    ker = nc.dram_tensor("kernels", (C, KK, H, W), F32, kind="ExternalInput").ap()
    out = nc.dram_tensor("out", (C, H, W), F16, kind="ExternalOutput").ap()

    s_pool = ctx.enter_context(tc.tile_pool(name="imgstage", bufs=2))
    e_pool = ctx.enter_context(tc.tile_pool(name="ext", bufs=2))
    kst_pool = ctx.enter_context(tc.tile_pool(name="kstage", bufs=6))
    kt_pool = ctx.enter_context(tc.tile_pool(name="kt", bufs=12))
    acc_pool = ctx.enter_context(tc.tile_pool(name="acc", bufs=2))
    tmp_pool = ctx.enter_context(tc.tile_pool(name="tmp", bufs=3))
    ps_pool = ctx.enter_context(tc.tile_pool(name="ps", bufs=4, space="PSUM"))
    id_pool = ctx.enter_context(tc.tile_pool(name="ident", bufs=1))

    # Shifted identities for TensorE partition shifts (as lhsT):
    # up[q, m] = 1 iff m == q+1  -> out[m] = rhs[m-1]   (row 4m-1 from 4p+3)
    # dn[q, m] = 1 iff m == q-1  -> out[m] = rhs[m+1]   (row 4m+4 from 4p)
    idc = id_pool.tile([P, 2, P + 1], F32, tag="idc")
    nc.gpsimd.memset(idc[:, :, :], 0.0)
    masks.make_identity(nc, idc[:, 0, 1 : P + 1], nomemset=True)
    masks.make_identity(nc, idc[:, 1, 0:P], nomemset=True)
    up = idc[:, 0, 0:P]
    dn = idc[:, 1, 1 : P + 1]

    # Tap order: mid-row taps (ki=1) first - they only need the cast of the
    # directly-loaded rows, not the TensorE boundary evacs, so the DVE chain
    # starts ~10us earlier. Load order matches.
    TAP_ORDER = [3, 4, 5, 0, 1, 2, 6, 7, 8]

    for c in range(C):
        kall = ker[c].rearrange("t (p b) x -> p t (b x)", b=RPB)

        # --- image mid rows: S[p, b, x] = img[c, 4p+b, x]  (f32) ---
        S = s_pool.tile([P, RPB, W], F32, tag="S")
        nc.scalar.dma_start(S[:, :, :], img[c].rearrange("(p b) x -> p b x", b=RPB))

        # Head fix (first channel): stream the first tap on the scalar ring
        # right behind the img DMA (sync ring boots later) so the SDMA
        # fleet is busy from the first microsecond, and its cast precedes
        # the ext cast on the in-order ACT queue.
        pre = {}
        if c == 0:
            t0 = TAP_ORDER[0]
            kst = kst_pool.tile([P, FW], F32, tag="kst")
            nc.scalar.dma_start(kst[:, :], kall[:, t0, :])
            kt = kt_pool.tile([P, FW], F16, tag="kt")
            nc.scalar.copy(kt[:, :], kst[:, :])
            pre[t0] = kt

        # --- ext: fp16 [128, 6, 514], zero pad cols ---
        ext = e_pool.tile([P, 6, EXT_W], F16, tag="ext")
        nc.gpsimd.memset(ext[:, :, 0:1], 0.0)
        nc.gpsimd.memset(ext[:, :, EXT_W - 1 : EXT_W], 0.0)
        nc.scalar.copy(ext[:, 1:5, 1 : W + 1], S[:, :, :])
        # boundary rows via TensorE partition shift, evac + cast on ACT
        ps_t = ps_pool.tile([P, W], F32, tag="ps")
        nc.tensor.matmul(ps_t[:, :], up, S[:, 3, :], start=True, stop=True)
        nc.scalar.copy(ext[:, 0, 1 : W + 1], ps_t[:, :])
        ps_b = ps_pool.tile([P, W], F32, tag="ps")
        nc.tensor.matmul(ps_b[:, :], dn, S[:, 0, :], start=True, stop=True)
        nc.scalar.copy(ext[:, 5, 1 : W + 1], ps_b[:, :])

        # --- kernel taps: stream one tap at a time, cast f32 -> fp16 ---
        last = c == C - 1

        acc = acc_pool.tile([P, FW], F16, tag="acc")
        out_c = out[c].rearrange("(p b) x -> p (b x)", b=RPB)
        order = TAP_ORDER[: KK - 1] if last else TAP_ORDER
        for n, t in enumerate(order):
            ki, kj = divmod(t, 3)
            if t in pre:
                kt = pre[t]
            else:
                kst = kst_pool.tile([P, FW], F32, tag="kst")
                nc.sync.dma_start(kst[:, :], kall[:, t, :])
                kt = kt_pool.tile([P, FW], F16, tag="kt")
                nc.scalar.copy(kt[:, :], kst[:, :])
            v = ext[:, ki : ki + RPB, kj : kj + W]
            ktap = _r3(kt[:, :])
            if n == 0:
                nc.vector.tensor_mul(_r3(acc[:, :]), v, ktap)
            else:
                tmp = tmp_pool.tile([P, FW], F16, tag="tmp")
                nc.vector.tensor_mul(_r3(tmp[:, :]), v, ktap)
                nc.vector.tensor_add(acc[:, :], acc[:, :], tmp[:, :])
        if not last:
            nc.gpsimd.dma_start(out_c, acc[:, :])
            continue
        # Last tap of the last channel: the post-last-DMA drain is the
        # kernel's tail, so run it as four cast-free quarter chains (mixed
        # f32*fp16 mults read the staged tap directly - no ACT cast hop)
        # and store each quarter from the idle SP ring as soon as its add
        # lands.
        t, ki, kj = KK - 1, 2, 2
        kqs = []
        for q in range(RPB):
            qsl = slice(q * W, (q + 1) * W)
            kq = kst_pool.tile([P, W], F32, tag="kstq")
            nc.sync.dma_start(kq[:, :], kall[:, t, qsl])
            kqs.append(kq)
        for q in range(RPB):
            qsl = slice(q * W, (q + 1) * W)
            tmq = tmp_pool.tile([P, W], F16, tag="tmpq")
            nc.vector.tensor_mul(tmq[:, :], ext[:, ki + q, kj : kj + W], kqs[q][:, :])
            nc.vector.tensor_add(acc[:, qsl], acc[:, qsl], tmq[:, :])
            # Quarter stores on the ACT HWDGE ring: idle at the tail, and a
            # store's wait here cannot block the sync ring's load issue.
            nc.scalar.dma_start(out_c[:, qsl], acc[:, qsl])


_NC_CACHE = []


def _build():
    nc = bacc.Bacc(
        "TRN2",
        target_bir_lowering=False,
        debug=False,
        enable_asserts=True,
        num_devices=1,
    )
    with tile.TileContext(nc) as tc:
        with ExitStack() as ctx:
            _emit(nc, tc, ctx)
    nc.compile()
    return nc


def kernel(img, kernels):
    """img: [8, 3, 512, 512] f32; kernels: [8, 3, 9, 512, 512] f32.
    Returns [8, 3, 512, 512] f32."""
    first_call = not _NC_CACHE
    if first_call:
        _NC_CACHE.append(_build())
    nc = _NC_CACHE[0]
    img = np.asarray(img, dtype=np.float32)
    kernels = np.asarray(kernels, dtype=np.float32)
    in_maps = [
        {
            "img": np.ascontiguousarray(img[b]),
            "kernels": np.ascontiguousarray(kernels[b]),
        }
        for b in range(NCORES)
    ]
    if first_call:
        # Warm-up execution: the very first run after a fresh NEFF
        # compile/load was observed to occasionally return stale output.
        run_bass_kernel_spmd(nc, in_maps, core_ids=list(range(NCORES)))
    res = run_bass_kernel_spmd(nc, in_maps, core_ids=list(range(NCORES)))
    return np.stack(
        [np.asarray(res.results[b]["out"], dtype=np.float32) for b in range(NCORES)],
        axis=0,
    )
